# revision 14
# baseline (speedup 1.0000x reference)
"""Multi-head attention (N=4, L=1024, E=1024, H=16, D=64) on 8 trn2 NeuronCores.

Sharding: core c = (batch n = c//2, head-group g = c%2); each core owns 8 heads
of one batch. Projections + attention + a partial output projection run on
device; the host sums the two per-batch partials and adds the output bias.

Device-side layout choices (all matmuls run as float32r at full PE rate):
  - host pre-transposes x (-> [embed, pos]) and weights so no PE transposes
    are needed on device;
  - q,k projections produce qT/kT in [head_dim, pos] layout, v in natural
    [pos, head_dim] layout with a ones-column appended per head;
  - scores are computed transposed (E^T[k,q] = K.Q^T), exp on ScalarE without
    max-subtraction (|E|/32 <~ 2, fp32-safe), and the [V|1]^T @ P^T matmul
    produces both the attention numerator and the softmax denominator;
  - the denominator row is broadcast across partitions via a DRAM bounce,
    reciprocal'd on VectorE and multiplied in to normalize.
"""

import sys
from contextlib import ExitStack

sys.path.insert(0, "/opt/trn_rl_repo")

import numpy as np

import concourse.bacc as bacc
import concourse.tile as tile
from concourse import mybir
from concourse.bass_utils import run_bass_kernel_spmd

EMBED = 1024
HEADS = 16
HEAD_DIM = 64
N_BATCH = 4
L = 1024
N_CORES = 8
HG = HEADS // 2          # heads per core
S = HG * HEAD_DIM        # per-core head-slice width (512)
KT = EMBED // 128        # k-tiles over the embed contraction dim (8)
MT = S // 128            # m-tiles over the head-slice dim (4)
F32 = mybir.dt.float32
F32R = mybir.dt.float32r
BF16 = mybir.dt.bfloat16
MM_DTYPE = "f32r"        # "f32r" | "bf16" — dtype of all matmul operands
SCALE = 1.0 / 32.0       # 1/sqrt(EMBED)

_CACHED = {}
DEBUG_DUMP = False


def _build(apply_mask: bool):
    MMD = F32R if MM_DTYPE == "f32r" else BF16
    nc = bacc.Bacc("TRN2", target_bir_lowering=False, debug=False,
                   num_devices=N_CORES)

    xqT = nc.dram_tensor("xqT", [EMBED, L], MMD, kind="ExternalInput").ap()
    xkT = nc.dram_tensor("xkT", [EMBED, L], MMD, kind="ExternalInput").ap()
    xvT = nc.dram_tensor("xvT", [EMBED, L], MMD, kind="ExternalInput").ap()
    wqT = nc.dram_tensor("wqT", [EMBED, S], MMD, kind="ExternalInput").ap()
    wkT = nc.dram_tensor("wkT", [EMBED, S], MMD, kind="ExternalInput").ap()
    wvT = nc.dram_tensor("wvT", [EMBED, S], MMD, kind="ExternalInput").ap()
    woT = nc.dram_tensor("woT", [S, EMBED], MMD, kind="ExternalInput").ap()
    bq_d = nc.dram_tensor("bq", [128, MT], F32, kind="ExternalInput").ap()
    bk_d = nc.dram_tensor("bk", [128, MT], F32, kind="ExternalInput").ap()
    bv_d = nc.dram_tensor("bv", [1, S], MMD, kind="ExternalInput").ap()
    ones_d = nc.dram_tensor("ones", [128, 128], MMD, kind="ExternalInput").ap()
    if apply_mask:
        mb_d = nc.dram_tensor("maskbT", [L, L], F32, kind="ExternalInput").ap()
    out_d = nc.dram_tensor("out_partial", [L, EMBED], F32,
                           kind="ExternalOutput").ap()
    dbg = {}
    if DEBUG_DUMP:
        for nm, shp in [("qT0", [128, L]), ("kT0", [128, L]), ("vb0", [128, HG * 65]),
                        ("e00", [128, L]), ("pt00", [128, L]), ("o0", [65, L]),
                        ("rcp0", [64, L]), ("xn0", [128, L])]:
            dbg[nm] = nc.dram_tensor(f"dbg_{nm}", shp, F32, kind="ExternalOutput").ap()

    with tile.TileContext(nc) as tc, ExitStack() as ctx:
        sb = ctx.enter_context(tc.tile_pool(name="sb", bufs=2))
        ps = ctx.enter_context(tc.tile_pool(name="ps", bufs=2, space="PSUM"))
        dr = ctx.enter_context(tc.tile_pool(name="dr", bufs=2, space="DRAM"))

        # constants / biases
        bq_sb = sb.tile([128, MT], F32, tag="bias")
        bk_sb = sb.tile([128, MT], F32, tag="bias")
        bv_sb = sb.tile([1, S], MMD, tag="bvrow")
        ones1 = sb.tile([1, 128], MMD, tag="ones1")
        nc.sync.dma_start(bq_sb[:], bq_d[:])
        nc.sync.dma_start(bk_sb[:], bk_d[:])
        nc.sync.dma_start(bv_sb[:], bv_d[:])
        nc.sync.dma_start(ones1[:], ones_d[0:1, :])

        # weight tiles (persist through their projection phase)
        p1_cm = tc.tile_pool(name="p1", bufs=2)
        p1 = p1_cm.__enter__()

        def load_w(name, src):
            tiles = []
            for k in range(KT):
                t = p1.tile([128, S], MMD, tag=f"w_{name}", bufs=KT)
                nc.sync.dma_start(t[:], src[k * 128:(k + 1) * 128, :])
                tiles.append(t)
            return tiles

        wq_t = load_w("q", wqT)
        wk_t = load_w("k", wkT)
        wv_t = load_w("v", wvT)

        def load_x(src):
            tiles = []
            for k in range(KT):
                t = p1.tile([128, L], MMD, tag="x", bufs=10)
                nc.sync.dma_start(t[:], src[k * 128:(k + 1) * 128, :])
                tiles.append(t)
            return tiles

        # ---- k / q projections -> transposed layout [head_dim_slice, pos]
        def proj_T(x_tiles, w_tiles, bias_sb, out_tag):
            outs = []
            for m in range(MT):
                p = ps.tile([128, L], F32, tag="pa")
                for ch in range(2):
                    cs = slice(ch * 512, (ch + 1) * 512)
                    for k in range(KT):
                        nc.tensor.matmul(
                            p[:, cs],
                            (w_tiles[k][:, m * 128:(m + 1) * 128]),
                            (x_tiles[k][:, cs]),
                            start=(k == 0), stop=(k == KT - 1))
                o = sb.tile([128, L], MMD, tag=out_tag, bufs=MT)
                nc.scalar.activation(o[:], p[:],
                                     mybir.ActivationFunctionType.Identity,
                                     bias=bias_sb[:, m:m + 1], scale=1.0)
                outs.append(o)
            return outs

        xk_tiles = load_x(xkT)
        kT_t = proj_T(xk_tiles, wk_t, bk_sb, "kT")
        xq_tiles = load_x(xqT)
        qT_t = proj_T(xq_tiles, wq_t, bq_sb, "qT")
        if DEBUG_DUMP:
            nc.sync.dma_start(dbg["qT0"][:, :], qT_t[0][:].bitcast(F32))
            nc.sync.dma_start(dbg["kT0"][:, :], kT_t[0][:].bitcast(F32))

        # ---- v projection -> natural layout [pos, head|ones] (stride 65)
        xv_tiles = load_x(xvT)
        v_t = []
        for mp in range(KT):  # 8 pos-tiles
            p = ps.tile([128, S], F32, tag="pb")
            for k in range(KT):
                nc.tensor.matmul(p[:], (xv_tiles[k][:, mp * 128:(mp + 1) * 128]),
                                 (wv_t[k][:]), start=(k == 0), stop=False)
            nc.tensor.matmul(p[:], (ones1[:]), (bv_sb[:]),
                             start=False, stop=True)
            vb = sb.tile([128, HG * 65], MMD, tag="vb", bufs=KT)
            vb3 = vb[:].rearrange("p (h d) -> p h d", h=HG)
            nc.sync.dma_start(vb3[:, :, 64:65], ones_d[:, 0:HG].rearrange("p (h d) -> p h d", d=1))
            nc.vector.tensor_copy(vb3[:, :, 0:64],
                                  p[:].rearrange("p (h d) -> p h d", h=HG))
            v_t.append(vb)
        if DEBUG_DUMP:
            nc.sync.dma_start(dbg["vb0"][:, :], v_t[0][:].bitcast(F32))

        p1_cm.__exit__(None, None, None)
        p2 = ctx.enter_context(tc.tile_pool(name="p2", bufs=2))

        if apply_mask:
            mb_t = []
            for k in range(KT):
                t = p2.tile([128, L], F32, tag="mb", bufs=KT)
                nc.sync.dma_start(t[:], mb_d[k * 128:(k + 1) * 128, :])
                mb_t.append(t)

        # ---- attention per head; xn tiles hold normalized att-out^T pairs
        xn_t = [sb.tile([128, L], MMD, tag="xn", bufs=MT, name=f"xn{i}")
                for i in range(MT)]
        for m in range(MT):  # head pair (2m, 2m+1), QK interleaved for
            pts = {0: [], 1: []}  # row-group concurrency on the PE
            for k in range(KT):
                for j in (0, 1):
                    h = 2 * m + j
                    rows = slice(j * 64, (j + 1) * 64)
                    e = ps.tile([128, L], F32, tag="pa", name=f"e{h}_{k}")
                    for ch in range(2):
                        cs = slice(ch * 512, (ch + 1) * 512)
                        nc.tensor.matmul(
                            e[:, cs],
                            (kT_t[m][rows, k * 128:(k + 1) * 128]),
                            (qT_t[m][rows, cs]),
                            start=True, stop=True)
                    pt = p2.tile([128, L], MMD, tag="pt", bufs=20,
                                 name=f"pt{h}_{k}")
                    if apply_mask:
                        es = p2.tile([128, L], F32, tag="es", bufs=2,
                                     name=f"es{h}_{k}")
                        nc.vector.tensor_add(es[:], e[:], mb_t[k][:])
                        nc.scalar.activation(pt[:], es[:],
                                             mybir.ActivationFunctionType.Exp,
                                             scale=SCALE)
                    else:
                        nc.scalar.activation(pt[:], e[:],
                                             mybir.ActivationFunctionType.Exp,
                                             scale=SCALE)
                    pts[j].append(pt)
            for j in (0, 1):
                h = 2 * m + j
                # [V|1]^T @ P^T -> numerator rows 0-63, denominator row 64
                o = ps.tile([65, L], F32, tag="pb", name=f"o{h}")
                for ch in range(2):
                    cs = slice(ch * 512, (ch + 1) * 512)
                    for k in range(KT):
                        nc.tensor.matmul(o[:, cs],
                                         (v_t[k][:, h * 65:(h + 1) * 65]),
                                         (pts[j][k][:, cs]),
                                         start=(k == 0), stop=(k == KT - 1))
                # normalize: denominator -> DRAM -> broadcast -> reciprocal
                den_row = p2.tile([65, L], F32, tag="rcprow", bufs=2,
                                  name=f"denrow{h}")
                nc.vector.tensor_copy(den_row[64:65, :], o[64:65, :])
                den = dr.tile([1, L], F32, tag="den", name=f"den{h}")
                nc.sync.dma_start(den[:], den_row[64:65, :])
                den_b = p2.tile([64, L], F32, tag="denb", bufs=2,
                                name=f"denb{h}")
                nc.sync.dma_start(den_b[:], den[:].to_broadcast((64, L)))
                rcp = p2.tile([64, L], F32, tag="rcp", bufs=2,
                              name=f"rcp{h}")
                nc.vector.reciprocal_approx_fast(rcp[:], den_b[:])
                if j == 0:
                    nc.vector.tensor_mul(xn_t[m][0:64, :], o[0:64, :], rcp[:])
                else:
                    xtmp = p2.tile([64, L], MMD, tag="xtmp", bufs=2,
                                   name=f"xtmp{h}")
                    nc.vector.tensor_mul(xtmp[:], o[0:64, :], rcp[:])
                    nc.sync.dma_start(xn_t[m][64:128, :], xtmp[:])

        if DEBUG_DUMP:
            nc.sync.dma_start(dbg["xn0"][:, :], xn_t[0][:].bitcast(F32))

        # ---- output projection partial: out[q, e] = xn^T.T @ woT
        wo_t = []
        for c in range(MT):
            t = p2.tile([128, EMBED], MMD, tag="wo", bufs=MT)
            nc.sync.dma_start(t[:], woT[c * 128:(c + 1) * 128, :])
            wo_t.append(t)
        for qt in range(KT):  # 8 q-tiles
            qs = slice(qt * 128, (qt + 1) * 128)
            for ec in range(2):
                es_ = slice(ec * 512, (ec + 1) * 512)
                f = ps.tile([128, 512], F32, tag="pa")
                for c in range(MT):
                    nc.tensor.matmul(f[:], (xn_t[c][:, qs]),
                                     (wo_t[c][:, es_]),
                                     start=(c == 0), stop=(c == MT - 1))
                os_ = p2.tile([128, 512], F32, tag="os", bufs=3)
                nc.scalar.copy(os_[:], f[:])
                nc.sync.dma_start(out_d[qs, es_], os_[:])

    nc.compile()
    return nc


def make_in_maps(values, keys, queries, mask, Wv, bv, Wk, bk, Wq, bq, Wo, bo):
    values = np.asarray(values, dtype=np.float32)
    keys = np.asarray(keys, dtype=np.float32)
    queries = np.asarray(queries, dtype=np.float32)
    mask = np.asarray(mask)
    Wv, bv = np.asarray(Wv, np.float32), np.asarray(bv, np.float32)
    Wk, bk = np.asarray(Wk, np.float32), np.asarray(bk, np.float32)
    Wq, bq = np.asarray(Wq, np.float32), np.asarray(bq, np.float32)
    Wo = np.asarray(Wo, np.float32)

    apply_mask = not bool(np.all(mask != 0))
    if MM_DTYPE == "bf16":
        import ml_dtypes
        mmd_np = ml_dtypes.bfloat16
    else:
        mmd_np = np.float32

    def ct(a):
        return np.ascontiguousarray(np.asarray(a, dtype=np.float32))

    def cm(a):
        return np.ascontiguousarray(np.asarray(a).astype(mmd_np))

    in_maps = []
    for c in range(N_CORES):
        n, g = c // 2, c % 2
        sl = slice(g * S, (g + 1) * S)
        m = {
            "xqT": cm(queries[n].T),
            "xkT": cm(keys[n].T),
            "xvT": cm(values[n].T),
            "wqT": cm(Wq[sl, :].T),
            "wkT": cm(Wk[sl, :].T),
            "wvT": cm(Wv[sl, :].T),
            "woT": cm(Wo[:, sl].T),
            "bq": ct(bq[sl].reshape(MT, 128).T),
            "bk": ct(bk[sl].reshape(MT, 128).T),
            "bv": cm(bv[sl].reshape(1, S)),
            "ones": np.ones((128, 128), mmd_np),
        }
        if apply_mask:
            mb = np.where(mask[n, 0] == 0, np.float32(-1e20), np.float32(0.0))
            m["maskbT"] = ct(mb.T)
        in_maps.append(m)
    return in_maps, apply_mask


def kernel(values, keys, queries, mask, Wv, bv, Wk, bk, Wq, bq, Wo, bo):
    in_maps, apply_mask = make_in_maps(values, keys, queries, mask, Wv, bv,
                                       Wk, bk, Wq, bq, Wo, bo)
    if apply_mask not in _CACHED:
        _CACHED[apply_mask] = _build(apply_mask)
    nc = _CACHED[apply_mask]

    res = run_bass_kernel_spmd(nc, in_maps, list(range(N_CORES))).results
    bo = np.asarray(bo, np.float32)
    out = np.empty((N_BATCH, L, EMBED), dtype=np.float32)
    for n in range(N_BATCH):
        out[n] = (res[2 * n]["out_partial"] + res[2 * n + 1]["out_partial"]
                  + bo[None, :])
    return out


# revision 15
# speedup vs baseline: 1.1921x; 1.1921x over previous
"""Multi-head attention (N=4, L=1024, E=1024, H=16, D=64) on 8 trn2 NeuronCores.

Sharding: core c = (batch n = c//2, head-group g = c%2); each core owns 8 heads
of one batch. Projections + attention + a partial output projection run on
device; the host sums the two per-batch partials and adds the output bias.

Device-side layout choices (all matmuls run as float32r at full PE rate):
  - host pre-transposes x (-> [embed, pos]) and weights so no PE transposes
    are needed on device;
  - q,k projections produce qT/kT in [head_dim, pos] layout, v in natural
    [pos, head_dim] layout with a ones-column appended per head;
  - scores are computed transposed (E^T[k,q] = K.Q^T), exp on ScalarE without
    max-subtraction (|E|/32 <~ 2, fp32-safe), and the [V|1]^T @ P^T matmul
    produces both the attention numerator and the softmax denominator;
  - the denominator row is broadcast across partitions via a DRAM bounce,
    reciprocal'd on VectorE and multiplied in to normalize.
"""

import sys
from contextlib import ExitStack

sys.path.insert(0, "/opt/trn_rl_repo")

import numpy as np

import concourse.bacc as bacc
import concourse.tile as tile
from concourse import mybir
from concourse.bass_utils import run_bass_kernel_spmd

EMBED = 1024
HEADS = 16
HEAD_DIM = 64
N_BATCH = 4
L = 1024
N_CORES = 8
HG = HEADS // 2          # heads per core
S = HG * HEAD_DIM        # per-core head-slice width (512)
KT = EMBED // 128        # k-tiles over the embed contraction dim (8)
MT = S // 128            # m-tiles over the head-slice dim (4)
F32 = mybir.dt.float32
F32R = mybir.dt.float32r
BF16 = mybir.dt.bfloat16
MM_DTYPE = "bf16"        # "f32r" | "bf16" — dtype of all matmul operands
SCALE = 1.0 / 32.0       # 1/sqrt(EMBED)

_CACHED = {}
DEBUG_DUMP = False


def _build(apply_mask: bool):
    MMD = F32R if MM_DTYPE == "f32r" else BF16
    nc = bacc.Bacc("TRN2", target_bir_lowering=False, debug=False,
                   num_devices=N_CORES)

    xqT = nc.dram_tensor("xqT", [EMBED, L], MMD, kind="ExternalInput").ap()
    xkT = nc.dram_tensor("xkT", [EMBED, L], MMD, kind="ExternalInput").ap()
    xvT = nc.dram_tensor("xvT", [EMBED, L], MMD, kind="ExternalInput").ap()
    wqT = nc.dram_tensor("wqT", [EMBED, S], MMD, kind="ExternalInput").ap()
    wkT = nc.dram_tensor("wkT", [EMBED, S], MMD, kind="ExternalInput").ap()
    wvT = nc.dram_tensor("wvT", [EMBED, S], MMD, kind="ExternalInput").ap()
    woT = nc.dram_tensor("woT", [S, EMBED], MMD, kind="ExternalInput").ap()
    bq_d = nc.dram_tensor("bq", [128, MT], F32, kind="ExternalInput").ap()
    bk_d = nc.dram_tensor("bk", [128, MT], F32, kind="ExternalInput").ap()
    bv_d = nc.dram_tensor("bv", [1, S], MMD, kind="ExternalInput").ap()
    ones_d = nc.dram_tensor("ones", [128, 128], MMD, kind="ExternalInput").ap()
    if apply_mask:
        mb_d = nc.dram_tensor("maskbT", [L, L], F32, kind="ExternalInput").ap()
    out_d = nc.dram_tensor("out_partial", [L, EMBED], F32,
                           kind="ExternalOutput").ap()
    dbg = {}
    if DEBUG_DUMP:
        for nm, shp in [("qT0", [128, L]), ("kT0", [128, L]), ("vb0", [128, HG * 65]),
                        ("e00", [128, L]), ("pt00", [128, L]), ("o0", [65, L]),
                        ("rcp0", [64, L]), ("xn0", [128, L])]:
            dbg[nm] = nc.dram_tensor(f"dbg_{nm}", shp, F32, kind="ExternalOutput").ap()

    with tile.TileContext(nc) as tc, ExitStack() as ctx:
        sb = ctx.enter_context(tc.tile_pool(name="sb", bufs=2))
        ps = ctx.enter_context(tc.tile_pool(name="ps", bufs=2, space="PSUM"))
        dr = ctx.enter_context(tc.tile_pool(name="dr", bufs=2, space="DRAM"))

        # constants / biases
        bq_sb = sb.tile([128, MT], F32, tag="bias")
        bk_sb = sb.tile([128, MT], F32, tag="bias")
        bv_sb = sb.tile([1, S], MMD, tag="bvrow")
        ones1 = sb.tile([1, 128], MMD, tag="ones1")
        nc.sync.dma_start(bq_sb[:], bq_d[:])
        nc.sync.dma_start(bk_sb[:], bk_d[:])
        nc.sync.dma_start(bv_sb[:], bv_d[:])
        nc.sync.dma_start(ones1[:], ones_d[0:1, :])

        # weight tiles (persist through their projection phase)
        p1_cm = tc.tile_pool(name="p1", bufs=2)
        p1 = p1_cm.__enter__()

        def load_w(name, src):
            tiles = []
            for k in range(KT):
                t = p1.tile([128, S], MMD, tag=f"w_{name}", bufs=KT)
                nc.sync.dma_start(t[:], src[k * 128:(k + 1) * 128, :])
                tiles.append(t)
            return tiles

        wq_t = load_w("q", wqT)
        wk_t = load_w("k", wkT)
        wv_t = load_w("v", wvT)

        def load_x(src):
            tiles = []
            for k in range(KT):
                t = p1.tile([128, L], MMD, tag="x", bufs=10)
                nc.sync.dma_start(t[:], src[k * 128:(k + 1) * 128, :])
                tiles.append(t)
            return tiles

        # ---- k / q projections -> transposed layout [head_dim_slice, pos]
        def proj_T(x_tiles, w_tiles, bias_sb, out_tag):
            outs = []
            for m in range(MT):
                p = ps.tile([128, L], F32, tag="pa")
                for ch in range(2):
                    cs = slice(ch * 512, (ch + 1) * 512)
                    for k in range(KT):
                        nc.tensor.matmul(
                            p[:, cs],
                            (w_tiles[k][:, m * 128:(m + 1) * 128]),
                            (x_tiles[k][:, cs]),
                            start=(k == 0), stop=(k == KT - 1))
                o = sb.tile([128, L], MMD, tag=out_tag, bufs=MT)
                nc.scalar.activation(o[:], p[:],
                                     mybir.ActivationFunctionType.Identity,
                                     bias=bias_sb[:, m:m + 1], scale=1.0)
                outs.append(o)
            return outs

        xk_tiles = load_x(xkT)
        kT_t = proj_T(xk_tiles, wk_t, bk_sb, "kT")
        xq_tiles = load_x(xqT)
        qT_t = proj_T(xq_tiles, wq_t, bq_sb, "qT")
        if DEBUG_DUMP:
            nc.sync.dma_start(dbg["qT0"][:, :], qT_t[0][:].bitcast(F32))
            nc.sync.dma_start(dbg["kT0"][:, :], kT_t[0][:].bitcast(F32))

        # ---- v projection -> natural layout [pos, head|ones] (stride 65)
        xv_tiles = load_x(xvT)
        v_t = []
        for mp in range(KT):  # 8 pos-tiles
            p = ps.tile([128, S], F32, tag="pb")
            for k in range(KT):
                nc.tensor.matmul(p[:], (xv_tiles[k][:, mp * 128:(mp + 1) * 128]),
                                 (wv_t[k][:]), start=(k == 0), stop=False)
            nc.tensor.matmul(p[:], (ones1[:]), (bv_sb[:]),
                             start=False, stop=True)
            vb = sb.tile([128, HG * 65], MMD, tag="vb", bufs=KT)
            vb3 = vb[:].rearrange("p (h d) -> p h d", h=HG)
            nc.sync.dma_start(vb3[:, :, 64:65], ones_d[:, 0:HG].rearrange("p (h d) -> p h d", d=1))
            nc.vector.tensor_copy(vb3[:, :, 0:64],
                                  p[:].rearrange("p (h d) -> p h d", h=HG))
            v_t.append(vb)
        if DEBUG_DUMP:
            nc.sync.dma_start(dbg["vb0"][:, :], v_t[0][:].bitcast(F32))

        p1_cm.__exit__(None, None, None)
        p2 = ctx.enter_context(tc.tile_pool(name="p2", bufs=2))

        if apply_mask:
            mb_t = []
            for k in range(KT):
                t = p2.tile([128, L], F32, tag="mb", bufs=KT)
                nc.sync.dma_start(t[:], mb_d[k * 128:(k + 1) * 128, :])
                mb_t.append(t)

        # ---- attention per head; xn tiles hold normalized att-out^T pairs
        xn_t = [sb.tile([128, L], MMD, tag="xn", bufs=MT, name=f"xn{i}")
                for i in range(MT)]
        for m in range(MT):  # head pair (2m, 2m+1), QK interleaved for
            pts = {0: [], 1: []}  # row-group concurrency on the PE
            for k in range(KT):
                for j in (0, 1):
                    h = 2 * m + j
                    rows = slice(j * 64, (j + 1) * 64)
                    e = ps.tile([128, L], F32, tag="pa", name=f"e{h}_{k}")
                    for ch in range(2):
                        cs = slice(ch * 512, (ch + 1) * 512)
                        nc.tensor.matmul(
                            e[:, cs],
                            (kT_t[m][rows, k * 128:(k + 1) * 128]),
                            (qT_t[m][rows, cs]),
                            start=True, stop=True)
                    pt = p2.tile([128, L], MMD, tag="pt", bufs=20,
                                 name=f"pt{h}_{k}")
                    if apply_mask:
                        es = p2.tile([128, L], F32, tag="es", bufs=2,
                                     name=f"es{h}_{k}")
                        nc.vector.tensor_add(es[:], e[:], mb_t[k][:])
                        nc.scalar.activation(pt[:], es[:],
                                             mybir.ActivationFunctionType.Exp,
                                             scale=SCALE)
                    else:
                        nc.scalar.activation(pt[:], e[:],
                                             mybir.ActivationFunctionType.Exp,
                                             scale=SCALE)
                    pts[j].append(pt)
            for j in (0, 1):
                h = 2 * m + j
                # [V|1]^T @ P^T -> numerator rows 0-63, denominator row 64
                o = ps.tile([65, L], F32, tag="pb", name=f"o{h}")
                for ch in range(2):
                    cs = slice(ch * 512, (ch + 1) * 512)
                    for k in range(KT):
                        nc.tensor.matmul(o[:, cs],
                                         (v_t[k][:, h * 65:(h + 1) * 65]),
                                         (pts[j][k][:, cs]),
                                         start=(k == 0), stop=(k == KT - 1))
                # normalize: denominator -> DRAM -> broadcast -> reciprocal
                den_row = p2.tile([65, L], F32, tag="rcprow", bufs=2,
                                  name=f"denrow{h}")
                nc.vector.tensor_copy(den_row[64:65, :], o[64:65, :])
                den = dr.tile([1, L], F32, tag="den", name=f"den{h}")
                nc.sync.dma_start(den[:], den_row[64:65, :])
                den_b = p2.tile([64, L], F32, tag="denb", bufs=2,
                                name=f"denb{h}")
                nc.sync.dma_start(den_b[:], den[:].to_broadcast((64, L)))
                rcp = p2.tile([64, L], F32, tag="rcp", bufs=2,
                              name=f"rcp{h}")
                nc.vector.reciprocal_approx_fast(rcp[:], den_b[:])
                if j == 0:
                    nc.vector.tensor_mul(xn_t[m][0:64, :], o[0:64, :], rcp[:])
                else:
                    xtmp = p2.tile([64, L], MMD, tag="xtmp", bufs=2,
                                   name=f"xtmp{h}")
                    nc.vector.tensor_mul(xtmp[:], o[0:64, :], rcp[:])
                    nc.sync.dma_start(xn_t[m][64:128, :], xtmp[:])

        if DEBUG_DUMP:
            nc.sync.dma_start(dbg["xn0"][:, :], xn_t[0][:].bitcast(F32))

        # ---- output projection partial: out[q, e] = xn^T.T @ woT
        wo_t = []
        for c in range(MT):
            t = p2.tile([128, EMBED], MMD, tag="wo", bufs=MT)
            nc.sync.dma_start(t[:], woT[c * 128:(c + 1) * 128, :])
            wo_t.append(t)
        for qt in range(KT):  # 8 q-tiles
            qs = slice(qt * 128, (qt + 1) * 128)
            for ec in range(2):
                es_ = slice(ec * 512, (ec + 1) * 512)
                f = ps.tile([128, 512], F32, tag="pa")
                for c in range(MT):
                    nc.tensor.matmul(f[:], (xn_t[c][:, qs]),
                                     (wo_t[c][:, es_]),
                                     start=(c == 0), stop=(c == MT - 1))
                os_ = p2.tile([128, 512], F32, tag="os", bufs=3)
                nc.scalar.copy(os_[:], f[:])
                nc.sync.dma_start(out_d[qs, es_], os_[:])

    nc.compile()
    return nc


def make_in_maps(values, keys, queries, mask, Wv, bv, Wk, bk, Wq, bq, Wo, bo):
    values = np.asarray(values, dtype=np.float32)
    keys = np.asarray(keys, dtype=np.float32)
    queries = np.asarray(queries, dtype=np.float32)
    mask = np.asarray(mask)
    Wv, bv = np.asarray(Wv, np.float32), np.asarray(bv, np.float32)
    Wk, bk = np.asarray(Wk, np.float32), np.asarray(bk, np.float32)
    Wq, bq = np.asarray(Wq, np.float32), np.asarray(bq, np.float32)
    Wo = np.asarray(Wo, np.float32)

    apply_mask = not bool(np.all(mask != 0))
    if MM_DTYPE == "bf16":
        import ml_dtypes
        mmd_np = ml_dtypes.bfloat16
    else:
        mmd_np = np.float32

    def ct(a):
        return np.ascontiguousarray(np.asarray(a, dtype=np.float32))

    def cm(a):
        return np.ascontiguousarray(np.asarray(a).astype(mmd_np))

    in_maps = []
    for c in range(N_CORES):
        n, g = c // 2, c % 2
        sl = slice(g * S, (g + 1) * S)
        m = {
            "xqT": cm(queries[n].T),
            "xkT": cm(keys[n].T),
            "xvT": cm(values[n].T),
            "wqT": cm(Wq[sl, :].T),
            "wkT": cm(Wk[sl, :].T),
            "wvT": cm(Wv[sl, :].T),
            "woT": cm(Wo[:, sl].T),
            "bq": ct(bq[sl].reshape(MT, 128).T),
            "bk": ct(bk[sl].reshape(MT, 128).T),
            "bv": cm(bv[sl].reshape(1, S)),
            "ones": np.ones((128, 128), mmd_np),
        }
        if apply_mask:
            mb = np.where(mask[n, 0] == 0, np.float32(-1e20), np.float32(0.0))
            m["maskbT"] = ct(mb.T)
        in_maps.append(m)
    return in_maps, apply_mask


def kernel(values, keys, queries, mask, Wv, bv, Wk, bk, Wq, bq, Wo, bo):
    in_maps, apply_mask = make_in_maps(values, keys, queries, mask, Wv, bv,
                                       Wk, bk, Wq, bq, Wo, bo)
    if apply_mask not in _CACHED:
        _CACHED[apply_mask] = _build(apply_mask)
    nc = _CACHED[apply_mask]

    res = run_bass_kernel_spmd(nc, in_maps, list(range(N_CORES))).results
    bo = np.asarray(bo, np.float32)
    out = np.empty((N_BATCH, L, EMBED), dtype=np.float32)
    for n in range(N_BATCH):
        out[n] = (res[2 * n]["out_partial"] + res[2 * n + 1]["out_partial"]
                  + bo[None, :])
    return out


# revision 17
# speedup vs baseline: 1.2292x; 1.0311x over previous
"""Multi-head attention (N=4, L=1024, E=1024, H=16, D=64) on 8 trn2 NeuronCores.

Sharding: core c = (batch n = c//2, head-group g = c%2); each core owns 8 heads
of one batch. Projections + attention + a partial output projection run on
device; the host sums the two per-batch partials and adds the output bias.

Device-side layout choices (all matmuls run as float32r at full PE rate):
  - host pre-transposes x (-> [embed, pos]) and weights so no PE transposes
    are needed on device;
  - q,k projections produce qT/kT in [head_dim, pos] layout, v in natural
    [pos, head_dim] layout with a ones-column appended per head;
  - scores are computed transposed (E^T[k,q] = K.Q^T), exp on ScalarE without
    max-subtraction (|E|/32 <~ 2, fp32-safe), and the [V|1]^T @ P^T matmul
    produces both the attention numerator and the softmax denominator;
  - the denominator row is broadcast across partitions via a DRAM bounce,
    reciprocal'd on VectorE and multiplied in to normalize.
"""

import sys
from contextlib import ExitStack

sys.path.insert(0, "/opt/trn_rl_repo")

import numpy as np

import concourse.bacc as bacc
import concourse.tile as tile
from concourse import mybir
from concourse.bass_utils import run_bass_kernel_spmd

EMBED = 1024
HEADS = 16
HEAD_DIM = 64
N_BATCH = 4
L = 1024
N_CORES = 8
HG = HEADS // 2          # heads per core
S = HG * HEAD_DIM        # per-core head-slice width (512)
KT = EMBED // 128        # k-tiles over the embed contraction dim (8)
MT = S // 128            # m-tiles over the head-slice dim (4)
F32 = mybir.dt.float32
F32R = mybir.dt.float32r
BF16 = mybir.dt.bfloat16
MM_DTYPE = "bf16"        # "f32r" | "bf16" — dtype of all matmul operands
SCALE = 1.0 / 32.0       # 1/sqrt(EMBED)

_CACHED = {}
DEBUG_DUMP = False


def _build(apply_mask: bool):
    MMD = F32R if MM_DTYPE == "f32r" else BF16
    nc = bacc.Bacc("TRN2", target_bir_lowering=False, debug=False,
                   num_devices=N_CORES)

    xqT = nc.dram_tensor("xqT", [EMBED, L], MMD, kind="ExternalInput").ap()
    xkT = nc.dram_tensor("xkT", [EMBED, L], MMD, kind="ExternalInput").ap()
    xvT = nc.dram_tensor("xvT", [EMBED, L], MMD, kind="ExternalInput").ap()
    wqT = nc.dram_tensor("wqT", [EMBED, S], MMD, kind="ExternalInput").ap()
    wkT = nc.dram_tensor("wkT", [EMBED, S], MMD, kind="ExternalInput").ap()
    wvT = nc.dram_tensor("wvT", [EMBED, S], MMD, kind="ExternalInput").ap()
    woT = nc.dram_tensor("woT", [S, EMBED], MMD, kind="ExternalInput").ap()
    bq_d = nc.dram_tensor("bq", [128, MT], F32, kind="ExternalInput").ap()
    bk_d = nc.dram_tensor("bk", [128, MT], F32, kind="ExternalInput").ap()
    bv_d = nc.dram_tensor("bv", [1, S], MMD, kind="ExternalInput").ap()
    ones_d = nc.dram_tensor("ones", [128, 128], MMD, kind="ExternalInput").ap()
    if apply_mask:
        mb_d = nc.dram_tensor("maskbT", [L, L], F32, kind="ExternalInput").ap()
    out_d = nc.dram_tensor("out_partial", [L, EMBED], F32,
                           kind="ExternalOutput").ap()
    dbg = {}
    if DEBUG_DUMP:
        for nm, shp in [("qT0", [128, L]), ("kT0", [128, L]), ("vb0", [128, HG * 65]),
                        ("e00", [128, L]), ("pt00", [128, L]), ("o0", [65, L]),
                        ("rcp0", [64, L]), ("xn0", [128, L])]:
            dbg[nm] = nc.dram_tensor(f"dbg_{nm}", shp, F32, kind="ExternalOutput").ap()

    with tile.TileContext(nc) as tc, ExitStack() as ctx:
        sb = ctx.enter_context(tc.tile_pool(name="sb", bufs=2))
        ps = ctx.enter_context(tc.tile_pool(name="ps", bufs=2, space="PSUM"))
        dr = ctx.enter_context(tc.tile_pool(name="dr", bufs=2, space="DRAM"))

        # constants / biases
        bq_sb = sb.tile([128, MT], F32, tag="bias")
        bk_sb = sb.tile([128, MT], F32, tag="bias")
        bv_sb = sb.tile([1, S], MMD, tag="bvrow")
        ones1 = sb.tile([1, 128], MMD, tag="ones1")
        nc.sync.dma_start(bq_sb[:], bq_d[:])
        nc.sync.dma_start(bk_sb[:], bk_d[:])
        nc.sync.dma_start(bv_sb[:], bv_d[:])
        nc.sync.dma_start(ones1[:], ones_d[0:1, :])

        # weight tiles (persist through their projection phase)
        p1_cm = tc.tile_pool(name="p1", bufs=2)
        p1 = p1_cm.__enter__()

        def load_w(name, src):
            tiles = []
            for k in range(KT):
                t = p1.tile([128, S], MMD, tag=f"w_{name}", bufs=KT)
                nc.sync.dma_start(t[:], src[k * 128:(k + 1) * 128, :])
                tiles.append(t)
            return tiles


        def load_x(src):
            tiles = []
            for k in range(KT):
                t = p1.tile([128, L], MMD, tag="x", bufs=10)
                nc.sync.dma_start(t[:], src[k * 128:(k + 1) * 128, :])
                tiles.append(t)
            return tiles

        # ---- k / q projections -> transposed layout [head_dim_slice, pos]
        def proj_T(x_tiles, w_tiles, bias_sb, out_tag):
            outs = []
            for m in range(MT):
                p = ps.tile([128, L], F32, tag="pa", bufs=3)
                for ch in range(2):
                    cs = slice(ch * 512, (ch + 1) * 512)
                    for k in range(KT):
                        nc.tensor.matmul(
                            p[:, cs],
                            (w_tiles[k][:, m * 128:(m + 1) * 128]),
                            (x_tiles[k][:, cs]),
                            start=(k == 0), stop=(k == KT - 1))
                o = sb.tile([128, L], MMD, tag=out_tag, bufs=MT)
                nc.vector.tensor_scalar_add(o[:], p[:], bias_sb[:, m:m + 1])
                outs.append(o)
            return outs

        # PE clock warmup: ~3.5us of dummy matmuls while the first DMAs land,
        # so the HAM un-throttles (1.2 -> 2.4 GHz) before real work starts.
        warm_ps = ps.tile([128, 512], F32, tag="pb", bufs=2)
        for i in range(8):
            nc.tensor.matmul(warm_ps[:, 0:512], (ones1[:]),
                             (bv_sb[0:1, 0:512]), start=True, stop=True)

        wk_t = load_w("k", wkT)
        xk_tiles = load_x(xkT)
        kT_t = proj_T(xk_tiles, wk_t, bk_sb, "kT")
        wq_t = load_w("q", wqT)
        xq_tiles = load_x(xqT)
        qT_t = proj_T(xq_tiles, wq_t, bq_sb, "qT")
        if DEBUG_DUMP:
            nc.sync.dma_start(dbg["qT0"][:, :], qT_t[0][:].bitcast(F32))
            nc.sync.dma_start(dbg["kT0"][:, :], kT_t[0][:].bitcast(F32))

        # ---- v projection -> natural layout [pos, head|ones] (stride 65)
        wv_t = load_w("v", wvT)
        xv_tiles = load_x(xvT)
        v_t = []
        for mp in range(KT):  # 8 pos-tiles
            p = ps.tile([128, S], F32, tag="pb", bufs=2)
            for k in range(KT):
                nc.tensor.matmul(p[:], (xv_tiles[k][:, mp * 128:(mp + 1) * 128]),
                                 (wv_t[k][:]), start=(k == 0), stop=False)
            nc.tensor.matmul(p[:], (ones1[:]), (bv_sb[:]),
                             start=False, stop=True)
            vb = sb.tile([128, HG * 65], MMD, tag="vb", bufs=KT)
            vb3 = vb[:].rearrange("p (h d) -> p h d", h=HG)
            nc.sync.dma_start(vb3[:, :, 64:65], ones_d[:, 0:HG].rearrange("p (h d) -> p h d", d=1))
            nc.vector.tensor_copy(vb3[:, :, 0:64],
                                  p[:].rearrange("p (h d) -> p h d", h=HG))
            v_t.append(vb)
        if DEBUG_DUMP:
            nc.sync.dma_start(dbg["vb0"][:, :], v_t[0][:].bitcast(F32))

        p1_cm.__exit__(None, None, None)
        p2 = ctx.enter_context(tc.tile_pool(name="p2", bufs=2))

        wo_t = []
        for c in range(MT):
            t = p2.tile([128, EMBED], MMD, tag="wo", bufs=MT, name=f"wo{c}")
            nc.sync.dma_start(t[:], woT[c * 128:(c + 1) * 128, :])
            wo_t.append(t)

        if apply_mask:
            mb_t = []
            for k in range(KT):
                t = p2.tile([128, L], F32, tag="mb", bufs=KT)
                nc.sync.dma_start(t[:], mb_d[k * 128:(k + 1) * 128, :])
                mb_t.append(t)

        # ---- attention per head; xn tiles hold normalized att-out^T pairs
        xn_t = [sb.tile([128, L], MMD, tag="xn", bufs=MT, name=f"xn{i}")
                for i in range(MT)]
        for m in range(MT):  # head pair (2m, 2m+1), QK interleaved for
            pts = {0: [], 1: []}  # row-group concurrency on the PE
            for k in range(KT):
                for j in (0, 1):
                    h = 2 * m + j
                    rows = slice(j * 64, (j + 1) * 64)
                    e = ps.tile([128, L], F32, tag="pa", bufs=3, name=f"e{h}_{k}")
                    for ch in range(2):
                        cs = slice(ch * 512, (ch + 1) * 512)
                        nc.tensor.matmul(
                            e[:, cs],
                            (kT_t[m][rows, k * 128:(k + 1) * 128]),
                            (qT_t[m][rows, cs]),
                            start=True, stop=True)
                    pt = p2.tile([128, L], MMD, tag="pt", bufs=20,
                                 name=f"pt{h}_{k}")
                    if apply_mask:
                        es = p2.tile([128, L], F32, tag="es", bufs=2,
                                     name=f"es{h}_{k}")
                        nc.vector.tensor_add(es[:], e[:], mb_t[k][:])
                        nc.scalar.activation(pt[:], es[:],
                                             mybir.ActivationFunctionType.Exp,
                                             scale=SCALE)
                    else:
                        nc.scalar.activation(pt[:], e[:],
                                             mybir.ActivationFunctionType.Exp,
                                             scale=SCALE)
                    pts[j].append(pt)
            for j in (0, 1):
                h = 2 * m + j
                # [V|1]^T @ P^T -> numerator rows 0-63, denominator row 64
                och = []
                for ch in range(2):
                    cs = slice(ch * 512, (ch + 1) * 512)
                    o = ps.tile([65, 512], F32, tag="pb", bufs=2,
                                name=f"o{h}_{ch}")
                    for k in range(KT):
                        nc.tensor.matmul(o[:],
                                         (v_t[k][:, h * 65:(h + 1) * 65]),
                                         (pts[j][k][:, cs]),
                                         start=(k == 0), stop=(k == KT - 1))
                    och.append(o)
                # normalize: denominator -> DRAM -> broadcast -> reciprocal
                den_row = p2.tile([65, L], F32, tag="rcprow", bufs=2,
                                  name=f"denrow{h}")
                for ch in range(2):
                    nc.vector.tensor_copy(den_row[64:65, ch * 512:(ch + 1) * 512],
                                          och[ch][64:65, :])
                den = dr.tile([1, L], F32, tag="den", name=f"den{h}")
                nc.sync.dma_start(den[:], den_row[64:65, :])
                den_b = p2.tile([64, L], F32, tag="denb", bufs=2,
                                name=f"denb{h}")
                nc.sync.dma_start(den_b[:], den[:].to_broadcast((64, L)))
                rcp = p2.tile([64, L], F32, tag="rcp", bufs=2,
                              name=f"rcp{h}")
                nc.vector.reciprocal_approx_fast(rcp[:], den_b[:])
                if j == 0:
                    for ch in range(2):
                        cs = slice(ch * 512, (ch + 1) * 512)
                        nc.vector.tensor_mul(xn_t[m][0:64, cs], och[ch][0:64, :],
                                             rcp[:, cs])
                else:
                    xtmp = p2.tile([64, L], MMD, tag="xtmp", bufs=2,
                                   name=f"xtmp{h}")
                    for ch in range(2):
                        cs = slice(ch * 512, (ch + 1) * 512)
                        nc.vector.tensor_mul(xtmp[:, cs], och[ch][0:64, :],
                                             rcp[:, cs])
                    nc.sync.dma_start(xn_t[m][64:128, :], xtmp[:])

        if DEBUG_DUMP:
            nc.sync.dma_start(dbg["xn0"][:, :], xn_t[0][:].bitcast(F32))

        # ---- output projection partial: out[q, e] = xn^T.T @ woT
        for qt in range(KT):  # 8 q-tiles
            qs = slice(qt * 128, (qt + 1) * 128)
            for ec in range(2):
                es_ = slice(ec * 512, (ec + 1) * 512)
                f = ps.tile([128, 512], F32, tag="pa", bufs=3)
                for c in range(MT):
                    nc.tensor.matmul(f[:], (xn_t[c][:, qs]),
                                     (wo_t[c][:, es_]),
                                     start=(c == 0), stop=(c == MT - 1))
                os_ = p2.tile([128, 512], F32, tag="os", bufs=3)
                nc.vector.tensor_copy(os_[:], f[:])
                nc.sync.dma_start(out_d[qs, es_], os_[:])

    nc.compile()
    return nc


def make_in_maps(values, keys, queries, mask, Wv, bv, Wk, bk, Wq, bq, Wo, bo):
    values = np.asarray(values, dtype=np.float32)
    keys = np.asarray(keys, dtype=np.float32)
    queries = np.asarray(queries, dtype=np.float32)
    mask = np.asarray(mask)
    Wv, bv = np.asarray(Wv, np.float32), np.asarray(bv, np.float32)
    Wk, bk = np.asarray(Wk, np.float32), np.asarray(bk, np.float32)
    Wq, bq = np.asarray(Wq, np.float32), np.asarray(bq, np.float32)
    Wo = np.asarray(Wo, np.float32)

    apply_mask = not bool(np.all(mask != 0))
    if MM_DTYPE == "bf16":
        import ml_dtypes
        mmd_np = ml_dtypes.bfloat16
    else:
        mmd_np = np.float32

    def ct(a):
        return np.ascontiguousarray(np.asarray(a, dtype=np.float32))

    def cm(a):
        return np.ascontiguousarray(np.asarray(a).astype(mmd_np))

    in_maps = []
    for c in range(N_CORES):
        n, g = c // 2, c % 2
        sl = slice(g * S, (g + 1) * S)
        m = {
            "xqT": cm(queries[n].T),
            "xkT": cm(keys[n].T),
            "xvT": cm(values[n].T),
            "wqT": cm(Wq[sl, :].T),
            "wkT": cm(Wk[sl, :].T),
            "wvT": cm(Wv[sl, :].T),
            "woT": cm(Wo[:, sl].T),
            "bq": ct(bq[sl].reshape(MT, 128).T),
            "bk": ct(bk[sl].reshape(MT, 128).T),
            "bv": cm(bv[sl].reshape(1, S)),
            "ones": np.ones((128, 128), mmd_np),
        }
        if apply_mask:
            mb = np.where(mask[n, 0] == 0, np.float32(-1e20), np.float32(0.0))
            m["maskbT"] = ct(mb.T)
        in_maps.append(m)
    return in_maps, apply_mask


def kernel(values, keys, queries, mask, Wv, bv, Wk, bk, Wq, bq, Wo, bo):
    in_maps, apply_mask = make_in_maps(values, keys, queries, mask, Wv, bv,
                                       Wk, bk, Wq, bq, Wo, bo)
    if apply_mask not in _CACHED:
        _CACHED[apply_mask] = _build(apply_mask)
    nc = _CACHED[apply_mask]

    res = run_bass_kernel_spmd(nc, in_maps, list(range(N_CORES))).results
    bo = np.asarray(bo, np.float32)
    out = np.empty((N_BATCH, L, EMBED), dtype=np.float32)
    for n in range(N_BATCH):
        out[n] = (res[2 * n]["out_partial"] + res[2 * n + 1]["out_partial"]
                  + bo[None, :])
    return out
